# revision 7
# baseline (speedup 1.0000x reference)
"""Trainium2 Bass kernel for nn_ConstraintsModule.

Reference math:
    m = preds[:, atoms]                                   # [B, N]
    body_rev[b,c,j] = pos_body[c,j] + m[b,j]*(neg_body-pos_body)[c,j]
    body_min[b,c]   = 1 - max_j body_rev[b,c,j]
    lb[b,n] = max_c body_min[b,c]*pos_head[c,n]
    ub[b,n] = 1 - max_c body_min[b,c]*neg_head[c,n]
    updated = clamp(m, min(lb,ub), max(lb,ub))
    out = preds with columns `atoms` replaced by updated

Device computes, per (batch row, head-atom, sign) "bin":
    bound[bin] = max over the bin's constraints c of
                 body_min[c] = min(m_{j in pos(c)}, (1-m)_{j in neg(c)}, 1)
via host-packed rows [m_pos..., (1-m)_neg..., 1.0 pads] (min-space form)
in bf16 (min/max never create values, so only the initial bf16 rounding
matters; measured rel err 3.9e-3 vs the 2e-2 gate).  The tiny elementwise
clamp (lb/ub vs m, O(B*N)) plus gather/scatter runs on the host, which
also owns the column -> (atom, sign) mapping, so device column order is
free to follow slot order.

Device structure (all compute on DVE; only DVE can reduce on this target):
  * G is split into uniform-width tier runs.  Each run is first narrowed
    by tensor_tensor min "fold" levels (bf16 2x mode, halving the width),
    then finished by tensor_reduce tails:
      - size-1 bins (the majority), packed first in the run: one reduce
        straight into output columns (body+head fused);
      - size>=2 bins, grouped by bucketed size class: one reduce per
        (run, class) into a bmin scratch arranged class-major.
  * One segment max-reduce per size class (over ALL runs/sides at once)
    produces the remaining output columns.

Sharding: bins follow their head atom; atom-sides are dealt round-robin
to the 8 cores so all cores share one SPMD program (counts padded to
ceil(n/8), dummy slots = all-zero rows -> bound 0, ignored by host).
"""

import sys
from collections import defaultdict
from contextlib import ExitStack

import numpy as np

if "/opt/trn_rl_repo" not in sys.path:
    sys.path.insert(0, "/opt/trn_rl_repo")

import ml_dtypes

import concourse.bacc as bacc
import concourse.tile as tile
from concourse import mybir
from concourse.bass_utils import run_bass_kernel_spmd

B = 128
C = 1024
N = 512
NCORES = 8

TIERS = (24, 32, 38)      # slot width tiers (per atom-side max constraint width)
SBUCK = (1, 2, 4, 8)      # bin-size buckets (1 kept exact -> fused)
CHUNK_ELEMS = 1500        # target per-partition elems per DMA piece
FOLD_MIN_ELEMS = 800      # keep folding while slots*width exceeds this

_TRACE = False
_LAST_RESULTS = None
_PROGRAM_CACHE: dict = {}


def _bucket(x):
    for v in SBUCK:
        if x <= v:
            return v
    raise ValueError(f"bin size {x} > {SBUCK[-1]}")


def _build_structure(bins):
    """bins: list of (side, atom, [cids], tier_idx, size).

    Layout:
      slot space: [run per tier] each = [fused bins][class-2 bins][class-4]..
      bmin space: class-major: [class2: run0, run1..][class4: ..]
      col space:  [run0 fused][run1 fused]..[class2 cols][class4 cols]..
    """
    gat = defaultdict(list)
    for side, atom, cids, t, s in bins:
        gat[(t, 1 if s == 1 else _bucket(s), side)].append((atom, cids))

    groups = []
    for (t, sb, side), members in sorted(gat.items()):
        cnt = -(-len(members) // NCORES)
        groups.append(dict(
            tier=t, sb=sb, side=side, cnt=cnt, members=members,
            w=TIERS[t], nslots=cnt * sb,
        ))

    soff = 0
    runs = []
    for t in range(len(TIERS)):
        tg = [g for g in groups if g["tier"] == t]
        if not tg:
            continue
        r = dict(t=t, w=TIERS[t], lo=soff, nfused=0, classes={})
        for g in tg:          # sorted: sb=1 first, then sb ascending
            g["soff"] = soff
            soff += g["nslots"]
            if g["sb"] == 1:
                r["nfused"] += g["cnt"]
            else:
                lo, n = r["classes"].get(g["sb"], (None, 0))
                if lo is None:
                    lo = g["soff"]
                r["classes"][g["sb"]] = (lo, n + g["nslots"])
        r["hi"] = soff
        runs.append(r)
    nslots = soff

    bptr = 0
    classes = {}
    for q in sorted({g["sb"] for g in groups if g["sb"] > 1}):
        chunks = []
        for ri, r in enumerate(runs):
            if q in r["classes"]:
                lo, n = r["classes"][q]
                chunks.append((ri, lo, n, bptr))
                bptr += n
        classes[q] = dict(q=q, chunks=chunks, boff0=chunks[0][3],
                          total=sum(c[2] for c in chunks))
    nbmin = bptr

    col = 0
    for r in runs:
        r["fcol"] = col
        col += r["nfused"]
    for q in sorted(classes):
        classes[q]["col"] = col
        col += classes[q]["total"] // q
    ncols = col

    for g in groups:
        ri = next(i for i, r in enumerate(runs) if r["t"] == g["tier"])
        r = runs[ri]
        if g["sb"] == 1:
            g["col"] = r["fcol"] + (g["soff"] - r["lo"])
        else:
            cl = classes[g["sb"]]
            ch = next(c for c in cl["chunks"] if c[0] == ri)
            boff = ch[3] + (g["soff"] - ch[1])
            g["boff"] = boff
            g["col"] = cl["col"] + (boff - cl["boff0"]) // g["sb"]

    core_members = [[] for _ in range(NCORES)]
    for gi, g in enumerate(groups):
        for j, (atom, cids) in enumerate(g["members"]):
            core_members[j % NCORES].append((gi, j // NCORES, atom, cids))

    return dict(groups=groups, runs=runs, classes=classes, nslots=nslots,
                nbmin=nbmin, ncols=ncols, core_members=core_members)


def _plan_dma(runs):
    pieces = []
    for r in runs:
        n = r["hi"] - r["lo"]
        w = r["w"]
        target = max(CHUNK_ELEMS // w, 8)
        k = max(1, round(n / target))
        step = -(-n // k)
        s = r["lo"]
        while s < r["hi"]:
            pieces.append((s, min(s + step, r["hi"]), w))
            s += step
    return pieces


def _build_program(skey, st, pieces):
    if skey in _PROGRAM_CACHE:
        return _PROGRAM_CACHE[skey]
    dt = mybir.dt
    bf = dt.bfloat16
    ncols = st["ncols"]
    nbmin = max(st["nbmin"], 1)

    nc = bacc.Bacc(
        "TRN2", target_bir_lowering=False, debug=False, enable_partition_id=False
    )
    c_ds = [
        nc.dram_tensor(f"c{i}", [B, (s1 - s0) * w], bf, kind="ExternalInput")
        for i, (s0, s1, w) in enumerate(pieces)
    ]
    out_d = nc.dram_tensor("bounds", [B, ncols], bf, kind="ExternalOutput")

    with ExitStack() as ctx:
        tc = ctx.enter_context(tile.TileContext(nc))
        pool = ctx.enter_context(tc.tile_pool(name="main", bufs=1))

        bounds = pool.tile([B, ncols], bf, tag="bounds")
        bmin = pool.tile([B, nbmin], bf, tag="bmin")

        run_tiles = []
        for r in st["runs"]:
            rt = pool.tile([B, (r["hi"] - r["lo"]) * r["w"]], bf,
                           name=f"run{r['lo']}", tag=f"run{r['lo']}")
            run_tiles.append(rt)

        dmaq = [nc.sync, nc.scalar]
        for i, (s0, s1, w) in enumerate(pieces):
            for r, rt in zip(st["runs"], run_tiles):
                if r["lo"] <= s0 and s1 <= r["hi"]:
                    dmaq[i % 2].dma_start(
                        rt[:, (s0 - r["lo"]) * w : (s1 - r["lo"]) * w],
                        c_ds[i].ap(),
                    )
                    break

        for ri, (r, rt) in enumerate(zip(st["runs"], run_tiles)):
            nrs = r["hi"] - r["lo"]
            w = r["w"]
            cur = rt[:].rearrange("p (s w) -> p s w", w=w)
            curw = w
            scratch = [None, None]
            pp = 0
            while curw > 2 and nrs * curw > FOLD_MIN_ELEMS:
                nh = (curw + 1) // 2
                if scratch[pp] is None:
                    scratch[pp] = pool.tile(
                        [B, nrs * nh], bf,
                        name=f"fs{r['lo']}_{pp}", tag=f"fs{r['lo']}_{pp}",
                    )
                nxt = scratch[pp][:, 0 : nrs * nh].rearrange(
                    "p (s w) -> p s w", w=nh
                )
                nc.vector.tensor_tensor(
                    nxt, cur[:, :, 0:nh], cur[:, :, curw - nh : curw],
                    op=mybir.AluOpType.min,
                )
                cur, curw = nxt, nh
                pp ^= 1
            if r["nfused"]:
                nc.vector.tensor_reduce(
                    bounds[:, r["fcol"] : r["fcol"] + r["nfused"]],
                    cur[:, 0 : r["nfused"], :],
                    axis=mybir.AxisListType.X, op=mybir.AluOpType.min,
                )
            for q in sorted(st["classes"]):
                for (cri, slot_lo, n, boff) in st["classes"][q]["chunks"]:
                    if cri != ri:
                        continue
                    rel = slot_lo - r["lo"]
                    nc.vector.tensor_reduce(
                        bmin[:, boff : boff + n],
                        cur[:, rel : rel + n, :],
                        axis=mybir.AxisListType.X, op=mybir.AluOpType.min,
                    )

        for q in sorted(st["classes"]):
            cl = st["classes"][q]
            ncl = cl["total"] // q
            seg = bmin[:, cl["boff0"] : cl["boff0"] + cl["total"]].rearrange(
                "p (a q) -> p a q", q=q
            )
            nc.vector.tensor_reduce(
                bounds[:, cl["col"] : cl["col"] + ncl], seg,
                axis=mybir.AxisListType.X, op=mybir.AluOpType.max,
            )

        nc.sync.dma_start(out_d.ap(), bounds[:])

    nc.compile()
    _PROGRAM_CACHE[skey] = nc
    return nc


def kernel(preds, pos_head, neg_head, pos_body, neg_body, atoms):
    global _LAST_RESULTS
    preds = np.ascontiguousarray(np.asarray(preds, dtype=np.float32))
    pos_head = np.asarray(pos_head)
    neg_head = np.asarray(neg_head)
    pos_body = np.asarray(pos_body)
    neg_body = np.asarray(neg_body)
    atoms_np = np.asarray(atoms).astype(np.int64)

    m = np.ascontiguousarray(preds[:, atoms_np].astype(np.float32))  # [B, N]
    one_m = np.float32(1.0) - m
    # m_ext columns: [0..N) m, [N..2N) 1-m, 2N: 1.0 (pad), 2N+1: 0.0 (dummy)
    m_ext = np.concatenate(
        [m, one_m, np.ones((B, 1), np.float32), np.zeros((B, 1), np.float32)],
        axis=1,
    )
    m_ext_bf = m_ext.astype(ml_dtypes.bfloat16)
    PAD1, PAD0 = 2 * N, 2 * N + 1

    pb = pos_body != 0
    nb_ = neg_body != 0
    k_c = (pb.sum(1) + nb_.sum(1)).astype(np.int64)
    body_js = [
        (np.nonzero(pb[c])[0], np.nonzero(nb_[c])[0]) for c in range(C)
    ]

    ph_atom = pos_head.argmax(1)
    ph_has = pos_head.max(1) > 0
    nh_atom = neg_head.argmax(1)
    nh_has = neg_head.max(1) > 0
    pos_bins = [[] for _ in range(N)]
    neg_bins = [[] for _ in range(N)]
    for c in np.nonzero(ph_has)[0]:
        pos_bins[ph_atom[c]].append(int(c))
    for c in np.nonzero(nh_has)[0]:
        neg_bins[nh_atom[c]].append(int(c))

    bins = []
    for n in range(N):
        for side, lst in (("P", pos_bins[n]), ("N", neg_bins[n])):
            if lst:
                kmax = max(k_c[c] for c in lst)
                t = next(i for i, w in enumerate(TIERS) if kmax <= w)
                bins.append((side, n, lst, t, len(lst)))

    st = _build_structure(bins)
    pieces = _plan_dma(st["runs"])
    skey = (
        tuple((g["tier"], g["side"], g["sb"], g["cnt"], g["col"], g["soff"],
               g.get("boff", -1)) for g in st["groups"]),
        tuple(pieces), st["ncols"],
    )
    nc = _build_program(skey, st, pieces)

    groups = st["groups"]
    in_maps = []
    percore_maps = []   # (cols, atoms, is_pos)
    for core in range(NCORES):
        idx = np.full((max(st["nslots"], 1), max(TIERS)), PAD0, np.int32)
        cl_, at_, sd_ = [], [], []
        for gi, pos_in_g, atom, cids in st["core_members"][core]:
            g = groups[gi]
            w = g["w"]
            base = g["soff"] + pos_in_g * g["sb"]
            for l, cid in enumerate(cids):
                jp, jn = body_js[cid]
                row = idx[base + l]
                row[: jp.size] = jp
                row[jp.size : jp.size + jn.size] = N + jn
                row[jp.size + jn.size : w] = PAD1
            cl_.append(g["col"] + pos_in_g)
            at_.append(atom)
            sd_.append(g["side"] == "P")
        im = {}
        for i, (s0, s1, w) in enumerate(pieces):
            im[f"c{i}"] = np.ascontiguousarray(m_ext_bf[:, idx[s0:s1, :w].ravel()])
        in_maps.append(im)
        percore_maps.append((np.array(cl_, np.int64), np.array(at_, np.int64),
                             np.array(sd_, bool)))

    res = run_bass_kernel_spmd(
        nc, in_maps, core_ids=list(range(NCORES)), trace=_TRACE
    )
    _LAST_RESULTS = res

    lb = np.zeros((B, N), np.float32)
    ubm = np.zeros((B, N), np.float32)
    for core in range(NCORES):
        bounds = np.asarray(res.results[core]["bounds"]).astype(np.float32)
        cols, ats, isp = percore_maps[core]
        if len(cols):
            if isp.any():
                lb[:, ats[isp]] = bounds[:, cols[isp]]
            if (~isp).any():
                ubm[:, ats[~isp]] = bounds[:, cols[~isp]]
    ub = np.float32(1.0) - ubm
    lo = np.minimum(lb, ub)
    hi = np.maximum(lb, ub)
    upd = np.maximum(lo, np.minimum(hi, m))
    out = preds.copy()
    out[:, atoms_np] = upd
    return out
